# revision 31
# baseline (speedup 1.0000x reference)
"""Trainium2 Bass kernel for nn_AttentionBlock (causal attention, column softmax).

Computation (reference):
    Q/K/V = X @ W + b  per batch b of X[4, 4096, 512]
    logits[t,s] = <q_t, k_s>, causal mask (s>t -> -inf),
    probs = softmax over t (per column s) / sqrt(512)
    out = X + probs @ V

Sharding: 8 cores = (batch b in 0..3) x (half h in 0..1). Within a batch the
32 key-blocks (128 rows each) are split between the two halves so that both
halves get one block of every "extent class" c (blocks 2c, 2c+1 share the
query window [256c, 4096)), giving an identical SPMD program on every core
with balanced causal work. Masks are data, not program structure.

All matmuls run in fp8(e4m3) with DoubleRow perf mode (2 fp8 weights/PE cell,
256-deep contraction per pass). Host pre-scales W2=Wk@Wq^T by 64 and Wv by 32
to keep fp8 mantissas in the normal range; the descales fold into the
activation writebacks. Per key-block: logits in 1024-wide PSUM chunks, exp on
ScalarE (bias c_s, accumulated row sums), then one DVE pass multiplies by
1/rowsum and emits normalized fp8 probs in [0,1]. V is written as fp8 once in
phase A. The AV matmuls contract fp8 probs x fp8 V pairwise over key-blocks
(odd counts padded with a zeroed probs block) and the fp32 PSUM result DMAs
straight to DRAM; the host applies 1/sqrt(512) and adds the residual.
"""
import sys
if "/opt/trn_rl_repo" not in sys.path:
    sys.path.insert(0, "/opt/trn_rl_repo")

import numpy as np
import ml_dtypes

import concourse.bass as bass  # noqa: F401  (bass must import before tile)
import concourse.tile as tile
from concourse import bacc, mybir
from concourse.bass_utils import run_bass_kernel_spmd

bf16 = ml_dtypes.bfloat16
f8 = ml_dtypes.float8_e4m3  # TRN fp8e4: max +-240
AFT = mybir.ActivationFunctionType
ALU = mybir.AluOpType
DR = mybir.MatmulPerfMode.DoubleRow

B, T, D = 4, 4096, 512      # K = V = D = 512
P = 128                     # partitions
NSLOT = 16                  # key blocks per core
ECH = 1024                  # exp chunk width (2 PSUM banks)
MCH = 512                   # matmul moving half-width (DoubleRow rhs <= 1024)
INV_SQRT_K = float(1.0 / np.sqrt(np.float32(D)))
W2S = 64.0                  # host pre-scale of W2 before fp8
ATS = 4.0                   # at = ATS*(A + Wq bk) in fp8
WVS = 32.0                  # host pre-scale of Wv before fp8
import os
_GPS = os.environ.get("KM_GP_SLOTS", "")
GP_SLOTS = frozenset(int(x) for x in _GPS.split(",") if x != "")  # normalize on GpSimd


def _build_program(reps=1, scratch_out=False, null_prog=False, with_bv=False):
    """scratch_out: write results to internal DRAM and expose a tiny external
    output — used only for device-time measurement (removes the 64MB/call
    host transfer). null_prog: same I/O signature, no work (overhead calib).
    """
    nc = bacc.Bacc("TRN2", target_bir_lowering=False, debug=False, num_devices=8)
    d8, dbf, df32 = mybir.dt.float8e4, mybir.dt.bfloat16, mybir.dt.float32

    XT = nc.dram_tensor("XT", [D, T], d8, kind="ExternalInput").ap()
    XST = nc.dram_tensor("XST", [D, 2048], d8, kind="ExternalInput").ap()
    W2 = nc.dram_tensor("W2", [D, D], d8, kind="ExternalInput").ap()   # 64*(Wk Wq^T)
    WV = nc.dram_tensor("WV", [D, D], d8, kind="ExternalInput").ap()   # 32*Wv
    QBK = nc.dram_tensor("QBK", [P, 4], df32, kind="ExternalInput").ap()  # 4*(Wq bk)
    CS = nc.dram_tensor("CS", [P, NSLOT], df32, kind="ExternalInput").ap()  # key bias
    MASKN = nc.dram_tensor("MASKN", [P, 2 * MCH], d8, kind="ExternalInput").ap()  # [0/-240 | 0]
    IDT = nc.dram_tensor("IDT", [P, 2 * P], d8, kind="ExternalInput").ap()  # [identity | 0]
    BVT = nc.dram_tensor("BVT", [P, D], dbf, kind="ExternalInput").ap() if with_bv else None
    if scratch_out or null_prog:
        OUT = nc.dram_tensor("OUTS", [T, D], dbf).ap()  # internal scratch
        OUT2 = nc.dram_tensor("OUT2", [P, 4], df32, kind="ExternalOutput").ap()
    else:
        OUT = nc.dram_tensor("OUT", [T, D], dbf, kind="ExternalOutput").ap()
        OUT2 = None

    if null_prog:
        with tile.TileContext(nc) as tc:
            with tc.tile_pool(name="nsb", bufs=1) as sb:
                t = sb.tile([P, 4], df32, tag="t")
                nc.sync.dma_start(t[:], QBK[:])
                nc.sync.dma_start(OUT2[:], t[:])
        nc.compile()
        return nc

    with tile.TileContext(nc) as tc:
        with tc.tile_pool(name="persist", bufs=1) as pp, \
             tc.tile_pool(name="pexp", bufs=2) as ep, \
             tc.tile_pool(name="small", bufs=3) as sp, \
             tc.tile_pool(name="lpsum", bufs=3, space="PSUM") as lp, \
             tc.tile_pool(name="cpsum", bufs=2, space="PSUM") as cp:

            # xt/at/vsf double-buffered by rep parity so rep r+1's loads and
            # phase A never wait on rep r's phase B/C readers.
            xt2 = [pp.tile([P, 4, T], d8, tag=f"xt{x}", name=f"xt{x}") for x in range(2)]
            at2 = [pp.tile([P, 4, 2048], d8, tag=f"at{x}", name=f"at{x}") for x in range(2)]
            vsf2 = [pp.tile([P, NSLOT, D], d8, tag=f"vsf{x}", name=f"vsf{x}") for x in range(2)]
            pf8 = pp.tile([P, NSLOT, T], d8, tag="pf8")     # normalized probs
            qbk = pp.tile([P, 4], df32, tag="qbk")
            cs = pp.tile([P, NSLOT], df32, tag="cs")
            maskn = pp.tile([P, 2, MCH], d8, tag="maskn")
            idt = pp.tile([P, 2, P], d8, tag="idt")
            bvt = pp.tile([P, D], dbf, tag="bvt", name="bvt") if with_bv else None

            # zero pads: odd AV pair-counts read a never-written probs region
            # of the next slot; zero it once (kept zero across reps).
            for i in range(0, NSLOT - 1, 2):
                nc.gpsimd.memset(pf8[:, i + 1:i + 2, 256 * i:256 * i + 256], 0.0)

            def one_rep(rep):
                xt = xt2[rep % 2]
                at = at2[rep % 2]
                vsf = vsf2[rep % 2]
                with tc.tile_pool(name=f"aph{rep}", bufs=1) as ap_:
                    aps = cp  # phase A shares the AV psum pool (same tag/shape)
                    xst = ap_.tile([P, 4, 2048], d8, tag="xst")  # X_sel^T: [d | s]
                    w2 = ap_.tile([P, 4, D], d8, tag="w2")       # [d | d']
                    wv = ap_.tile([P, 4, D], d8, tag="wv")

                    # phase-A-critical first: xst + w2 land on the 8 DMA queues
                    # in parallel; the big xt (phase B only) goes last.
                    for db in range(4):
                        nc.sync.dma_start(xst[:, db:db + 1, :], XST[P * db:P * (db + 1), :])
                        nc.sync.dma_start(w2[:, db:db + 1, :], W2[P * db:P * (db + 1), :])
                    for db in range(4):
                        nc.sync.dma_start(wv[:, db:db + 1, :], WV[P * db:P * (db + 1), :])
                    nc.sync.dma_start(qbk[:], QBK[:])
                    nc.sync.dma_start(cs[:], CS[:])
                    nc.sync.dma_start(maskn[:, 0:1, :], MASKN[:, 0:MCH])
                    nc.sync.dma_start(maskn[:, 1:2, :], MASKN[:, MCH:])
                    nc.sync.dma_start(idt[:, 0:1, :], IDT[:, 0:P])
                    nc.sync.dma_start(idt[:, 1:2, :], IDT[:, P:])
                    if with_bv:
                        nc.sync.dma_start(bvt[:], BVT[:])
                    for db in range(4):
                        nc.sync.dma_start(xt[:, db:db + 1, :], XT[P * db:P * (db + 1), :])

                    # ATS*A^T[d', s] = ATS * ((Wq Wk^T)[d', d] X_sel^T[d, s] + (Wq bk)[d'])
                    for kb in range(4):
                        for sc in range(4):
                            ps = aps.tile([P, MCH], df32, tag="avp")
                            for x, dbp in enumerate((0, 2)):
                                nc.tensor.matmul(
                                    ps[:],
                                    w2[:, dbp:dbp + 2, P * kb:P * (kb + 1)],
                                    xst[:, dbp:dbp + 2, MCH * sc:MCH * (sc + 1)],
                                    start=(x == 0), stop=(x == 1), perf_mode=DR,
                                )
                            nc.vector.tensor_scalar(
                                out=at[:, kb:kb + 1, MCH * sc:MCH * (sc + 1)],
                                in0=ps[:], scalar1=ATS / W2S,
                                scalar2=qbk[:, kb:kb + 1],
                                op0=ALU.mult, op1=ALU.add,
                            )
                    # V[s, v] = sum_d X_sel[s, d] Wv[d, v]  (+ bv broadcast)
                    for i in range(NSLOT):
                        ps = aps.tile([P, MCH], df32, tag="avp")
                        for x, dbp in enumerate((0, 2)):
                            nc.tensor.matmul(
                                ps[:],
                                xst[:, dbp:dbp + 2, P * i:P * (i + 1)],
                                wv[:, dbp:dbp + 2, :],
                                start=(x == 0), stop=(x == 1), perf_mode=DR,
                            )
                        if with_bv:
                            tv = sp.tile([P, D], dbf, tag="tv")
                            nc.vector.tensor_scalar_mul(tv[:], ps[:], 1.0 / WVS)
                            nc.vector.tensor_add(vsf[:, i:i + 1, :], tv[:], bvt[:])
                        else:
                            nc.vector.tensor_scalar_mul(vsf[:, i:i + 1, :], ps[:], 1.0 / WVS)

                # Phase B: per slot logits -> exp -> row sums -> normalized fp8
                for i in range(NSLOT):
                    c0 = 256 * i
                    chunks = [(t0, min(ECH, T - t0)) for t0 in range(c0, T, ECH)]
                    nch = len(chunks)
                    sums = sp.tile([P, 4], df32, tag="sums")
                    pexp = ep.tile([P, T], dbf, tag="pexp")
                    for g0 in range(0, nch, 2):
                        grp = chunks[g0:g0 + 2]
                        pss = []
                        for _g in range(len(grp)):
                            lgt = lp.tile([P, ECH], df32, tag="lg", name=f"lg{_g}")
                            pss.append(lgt)
                        # kbp outer so the at stationary loads once per 2 chunks
                        for x, kbp in enumerate((0, 2)):
                            for el, (ps, (t0, w)) in enumerate(zip(pss, grp)):
                                diag = (g0 + el == 0)
                                for h0 in range(0, w, MCH):
                                    hw = min(MCH, w - h0)
                                    nc.tensor.matmul(
                                        ps[:, h0:h0 + hw],
                                        at[:, kbp:kbp + 2, P * i:P * (i + 1)],
                                        xt[:, kbp:kbp + 2, t0 + h0:t0 + h0 + hw],
                                        start=(x == 0),
                                        stop=(x == 1) and not (diag and h0 == 0),
                                        perf_mode=DR,
                                    )
                                if x == 1 and diag:
                                    # diagonal block: add -240 to masked entries
                                    # (DoubleRow with a zero second plane)
                                    mw = min(MCH, w)
                                    nc.tensor.matmul(
                                        ps[:, 0:mw], idt[:, :, 0:P],
                                        maskn[:, :, 0:mw],
                                        start=False, stop=True, perf_mode=DR,
                                    )
                        for el, (ps, (t0, w)) in enumerate(zip(pss, grp)):
                            e = g0 + el
                            nc.scalar.activation(
                                pexp[:, t0:t0 + w], ps[:, 0:w],
                                AFT.Exp, scale=1.0 / ATS, bias=cs[:, i:i + 1],
                                accum_out=sums[:, e:e + 1],
                            )
                    den = sp.tile([P, 1], df32, tag="den")
                    nc.vector.tensor_reduce(den[:], sums[:, 0:nch],
                                            axis=mybir.AxisListType.X, op=ALU.add)
                    r2 = sp.tile([P, 1], df32, tag="r2")
                    nc.vector.reciprocal(r2[:], den[:])
                    eng = nc.gpsimd if i in GP_SLOTS else nc.vector
                    eng.tensor_scalar_mul(pf8[:, i:i + 1, c0:T], pexp[:, c0:T], r2[:])

                    # Phase C interleaved: t-blocks 2i, 2i+1 need only slots 0..i,
                    # so their AV matmuls can fill PE gaps during later softmaxes.
                    for tau in (2 * i, 2 * i + 1):
                        n = i + 1  # slots with window start <= 128*tau
                        jps = list(range(0, n, 2))  # odd n: last pair hits zero pad
                        ps = cp.tile([P, D], df32, tag="avp")
                        for idx, jp in enumerate(jps):
                            nc.tensor.matmul(
                                ps[:],
                                pf8[:, jp:jp + 2, P * tau:P * (tau + 1)],
                                vsf[:, jp:jp + 2, :],
                                start=(idx == 0), stop=(idx == len(jps) - 1),
                                perf_mode=DR,
                            )
                        st = sp.tile([P, D], dbf, tag="st")
                        nc.vector.tensor_scalar_mul(st[:], ps[:], INV_SQRT_K)
                        nc.sync.dma_start(OUT[P * tau:P * (tau + 1), :], st[:])

            for rep in range(reps):
                one_rep(rep)

            if OUT2 is not None:
                fin = sp.tile([P, 4], df32, tag="fin")
                nc.gpsimd.memset(fin[:], 0.0)
                nc.sync.dma_start(OUT2[:], fin[:])

    nc.compile()
    return nc


_PROGRAMS = {}


def _get_program(with_bv=False):
    if with_bv not in _PROGRAMS:
        _PROGRAMS[with_bv] = _build_program(with_bv=with_bv)
    return _PROGRAMS[with_bv]


def _f8(x, scale=1.0):
    return np.clip(np.asarray(x, np.float32) * scale, -240.0, 240.0).astype(f8)


def _core_inputs(X, W2_b, Wv_b, QBK_h, BV_b, masks, wkbq, bkbq, b, h):
    """Per-core input map for core (b, h)."""
    Xb = X[b]
    XTb = _f8(np.ascontiguousarray(Xb.T))
    sel = Xb.reshape(16, 2, P, D)[:, h].reshape(2048, D)
    XSTb = _f8(np.ascontiguousarray(sel.T))
    # per-key-row logit bias c_s = (x_s Wk).bq + bk.bq, [2048] -> [128, 16]
    cvec = sel.astype(np.float64) @ wkbq + bkbq
    CS_h = np.ascontiguousarray(cvec.reshape(NSLOT, P).T).astype(np.float32)
    m = {
        "XT": XTb, "XST": XSTb,
        "W2": W2_b, "WV": Wv_b,
        "QBK": QBK_h, "CS": CS_h,
        "MASKN": masks[h], "IDT": masks[2],
    }
    if BV_b is not None:
        m["BVT"] = BV_b
    return m


def _prep_shared(Wk, bk, Wq, bq, Wv, bv):
    Wk64 = np.asarray(Wk, np.float64)
    Wq64 = np.asarray(Wq, np.float64)
    W2_b = _f8(Wk64 @ Wq64.T, W2S)                               # lhsT for A^T
    Wv_b = _f8(np.asarray(Wv), WVS)
    qbk = Wq64 @ np.asarray(bk, np.float64)                      # [512]
    QBK_h = np.ascontiguousarray(qbk.reshape(4, P).T * ATS).astype(np.float32)
    wkbq = Wk64 @ np.asarray(bq, np.float64)                     # [512]
    bkbq = float(np.asarray(bk, np.float64) @ np.asarray(bq, np.float64))
    if np.any(np.asarray(bv) != 0):
        BV_b = np.tile(np.asarray(bv).astype(bf16)[None, :], (P, 1))
    else:
        BV_b = None
    s_loc = np.arange(P)[:, None]
    t_loc = np.arange(MCH)[None, :]
    masks = []  # [maskn_h0 | zeros, maskn_h1 | zeros, identity | zeros]
    for h in range(2):
        mn = np.zeros((P, 2 * MCH), np.float32)
        mn[:, :MCH] = np.where(t_loc >= P * h + s_loc, 0.0, -240.0)
        masks.append(mn.astype(f8))
    idz = np.zeros((P, 2 * P), np.float32)
    idz[:, :P] = np.eye(P)
    masks.append(idz.astype(f8))
    return W2_b, Wv_b, QBK_h, BV_b, masks, wkbq, bkbq


def kernel(minibatch, Wk, bk, Wq, bq, Wv, bv):
    X = np.asarray(minibatch, dtype=np.float32)
    shared = _prep_shared(Wk, bk, Wq, bq, Wv, bv)
    nc = _get_program(with_bv=shared[3] is not None)
    in_maps = [
        _core_inputs(X, *shared, b, h)
        for b in range(B) for h in range(2)
    ]
    last_exc = None
    for attempt in range(4):
        try:
            res = run_bass_kernel_spmd(nc, in_maps, list(range(2 * B)))
        except Exception as exc:  # transient device wedge — retry
            last_exc = exc
            continue
        out = X.copy()
        for b in range(B):
            out[b] += res.results[2 * b]["OUT"].astype(np.float32)
            out[b] += res.results[2 * b + 1]["OUT"].astype(np.float32)
        # transient device faults can surface as NaN/garbage — retry
        if not np.isnan(out).any() and np.abs(out).max() < 1e4:
            return out
    if last_exc is not None:
        raise last_exc
    return out


# revision 32
# speedup vs baseline: 2.5740x; 2.5740x over previous
"""Trainium2 Bass kernel for nn_AttentionBlock (causal attention, column softmax).

Computation (reference):
    Q/K/V = X @ W + b  per batch b of X[4, 4096, 512]
    logits[t,s] = <q_t, k_s>, causal mask (s>t -> -inf),
    probs = softmax over t (per column s) / sqrt(512)
    out = X + probs @ V

Sharding: 8 cores = (batch b in 0..3) x (half h in 0..1). Within a batch the
32 key-blocks (128 rows each) are split between the two halves so that both
halves get one block of every "extent class" c (blocks 2c, 2c+1 share the
query window [256c, 4096)), giving an identical SPMD program on every core
with balanced causal work. Masks are data, not program structure.

All matmuls run in fp8(e4m3) with DoubleRow perf mode (2 fp8 weights/PE cell,
256-deep contraction per pass). Host pre-scales W2=Wk@Wq^T by 64 and Wv by 32
to keep fp8 mantissas in the normal range; the descales fold into the
activation writebacks. Per key-block: logits in 1024-wide PSUM chunks, exp on
ScalarE (bias c_s, accumulated row sums), then one DVE pass multiplies by
1/rowsum and emits normalized fp8 probs in [0,1]. V is written as fp8 once in
phase A. The AV matmuls contract fp8 probs x fp8 V pairwise over key-blocks
(odd counts padded with a zeroed probs block) and the fp32 PSUM result DMAs
straight to DRAM; the host applies 1/sqrt(512) and adds the residual.
"""
import sys
if "/opt/trn_rl_repo" not in sys.path:
    sys.path.insert(0, "/opt/trn_rl_repo")

import numpy as np
import ml_dtypes

import concourse.bass as bass  # noqa: F401  (bass must import before tile)
import concourse.tile as tile
from concourse import bacc, mybir
from concourse.bass_utils import run_bass_kernel_spmd

bf16 = ml_dtypes.bfloat16
f8 = ml_dtypes.float8_e4m3  # TRN fp8e4: max +-240
AFT = mybir.ActivationFunctionType
ALU = mybir.AluOpType
DR = mybir.MatmulPerfMode.DoubleRow

B, T, D = 4, 4096, 512      # K = V = D = 512
P = 128                     # partitions
NSLOT = 16                  # key blocks per core
ECH = 1024                  # exp chunk width (2 PSUM banks)
MCH = 512                   # matmul moving half-width (DoubleRow rhs <= 1024)
INV_SQRT_K = float(1.0 / np.sqrt(np.float32(D)))
W2S = 64.0                  # host pre-scale of W2 before fp8
ATS = 4.0                   # at = ATS*(A + Wq bk) in fp8
WVS = 32.0                  # host pre-scale of Wv before fp8
import os
_GPS = os.environ.get("KM_GP_SLOTS", "")
GP_SLOTS = frozenset(int(x) for x in _GPS.split(",") if x != "")  # normalize on GpSimd


def _build_program(reps=1, scratch_out=False, null_prog=False, with_bv=False):
    """scratch_out: write results to internal DRAM and expose a tiny external
    output — used only for device-time measurement (removes the 64MB/call
    host transfer). null_prog: same I/O signature, no work (overhead calib).
    """
    nc = bacc.Bacc("TRN2", target_bir_lowering=False, debug=False, num_devices=8)
    d8, dbf, df32 = mybir.dt.float8e4, mybir.dt.bfloat16, mybir.dt.float32

    XT = nc.dram_tensor("XT", [D, T], d8, kind="ExternalInput").ap()
    XST = nc.dram_tensor("XST", [D, 2048], d8, kind="ExternalInput").ap()
    W2 = nc.dram_tensor("W2", [D, D], d8, kind="ExternalInput").ap()   # 64*(Wk Wq^T)
    WV = nc.dram_tensor("WV", [D, D], d8, kind="ExternalInput").ap()   # 32*Wv
    QBK = nc.dram_tensor("QBK", [P, 4], df32, kind="ExternalInput").ap()  # 4*(Wq bk)
    CS = nc.dram_tensor("CS", [P, NSLOT], df32, kind="ExternalInput").ap()  # key bias
    MASKN = nc.dram_tensor("MASKN", [P, 2 * MCH], d8, kind="ExternalInput").ap()  # [0/-240 | 0]
    IDT = nc.dram_tensor("IDT", [P, 2 * P], d8, kind="ExternalInput").ap()  # [identity | 0]
    BVT = nc.dram_tensor("BVT", [P, D], dbf, kind="ExternalInput").ap() if with_bv else None
    if scratch_out or null_prog:
        OUT = nc.dram_tensor("OUTS", [T, D], dbf).ap()  # internal scratch
        OUT2 = nc.dram_tensor("OUT2", [P, 4], df32, kind="ExternalOutput").ap()
    else:
        OUT = nc.dram_tensor("OUT", [T, D], dbf, kind="ExternalOutput").ap()
        OUT2 = None

    if null_prog:
        with tile.TileContext(nc) as tc:
            with tc.tile_pool(name="nsb", bufs=1) as sb:
                t = sb.tile([P, 4], df32, tag="t")
                nc.sync.dma_start(t[:], QBK[:])
                nc.sync.dma_start(OUT2[:], t[:])
        nc.compile()
        return nc

    with tile.TileContext(nc) as tc:
        with tc.tile_pool(name="persist", bufs=1) as pp, \
             tc.tile_pool(name="pexp", bufs=2) as ep, \
             tc.tile_pool(name="small", bufs=3) as sp, \
             tc.tile_pool(name="lpsum", bufs=3, space="PSUM") as lp, \
             tc.tile_pool(name="cpsum", bufs=2, space="PSUM") as cp:

            # xt/at/vsf double-buffered by rep parity so rep r+1's loads and
            # phase A never wait on rep r's phase B/C readers.
            xt2 = [pp.tile([P, 4, T], d8, tag=f"xt{x}", name=f"xt{x}") for x in range(2)]
            at2 = [pp.tile([P, 4, 2048], d8, tag=f"at{x}", name=f"at{x}") for x in range(2)]
            vsf2 = [pp.tile([P, NSLOT, D], d8, tag=f"vsf{x}", name=f"vsf{x}") for x in range(2)]
            pf8 = pp.tile([P, NSLOT, T], d8, tag="pf8")     # normalized probs
            qbk = pp.tile([P, 4], df32, tag="qbk")
            cs = pp.tile([P, NSLOT], df32, tag="cs")
            maskn = pp.tile([P, 2, MCH], d8, tag="maskn")
            idt = pp.tile([P, 2, P], d8, tag="idt")
            bvt = pp.tile([P, D], dbf, tag="bvt", name="bvt") if with_bv else None

            # zero pads: odd AV pair-counts read a never-written probs region
            # of the next slot; zero it once (kept zero across reps).
            for i in range(0, NSLOT - 1, 2):
                nc.gpsimd.memset(pf8[:, i + 1:i + 2, 256 * i:256 * i + 256], 0.0)

            def one_rep(rep):
                xt = xt2[rep % 2]
                at = at2[rep % 2]
                vsf = vsf2[rep % 2]
                with tc.tile_pool(name=f"aph{rep}", bufs=1) as ap_:
                    aps = cp  # phase A shares the AV psum pool (same tag/shape)
                    xst = ap_.tile([P, 4, 2048], d8, tag="xst")  # X_sel^T: [d | s]
                    w2 = ap_.tile([P, 4, D], d8, tag="w2")       # [d | d']
                    wv = ap_.tile([P, 4, D], d8, tag="wv")

                    # phase-A-critical first: xst + w2 land on the 8 DMA queues
                    # in parallel; the big xt (phase B only) goes last.
                    for db in range(4):
                        nc.sync.dma_start(xst[:, db:db + 1, :], XST[P * db:P * (db + 1), :])
                        nc.sync.dma_start(w2[:, db:db + 1, :], W2[P * db:P * (db + 1), :])
                    for db in range(4):
                        nc.sync.dma_start(wv[:, db:db + 1, :], WV[P * db:P * (db + 1), :])
                    nc.sync.dma_start(qbk[:], QBK[:])
                    nc.sync.dma_start(cs[:], CS[:])
                    nc.sync.dma_start(maskn[:, 0:1, :], MASKN[:, 0:MCH])
                    nc.sync.dma_start(maskn[:, 1:2, :], MASKN[:, MCH:])
                    nc.sync.dma_start(idt[:, 0:1, :], IDT[:, 0:P])
                    nc.sync.dma_start(idt[:, 1:2, :], IDT[:, P:])
                    if with_bv:
                        nc.sync.dma_start(bvt[:], BVT[:])
                    for db in range(4):
                        nc.sync.dma_start(xt[:, db:db + 1, :], XT[P * db:P * (db + 1), :])

                    # ATS*A^T[d', s] = ATS * ((Wq Wk^T)[d', d] X_sel^T[d, s] + (Wq bk)[d'])
                    for kb in range(4):
                        for sc in range(4):
                            ps = aps.tile([P, MCH], df32, tag="avp")
                            for x, dbp in enumerate((0, 2)):
                                nc.tensor.matmul(
                                    ps[:],
                                    w2[:, dbp:dbp + 2, P * kb:P * (kb + 1)],
                                    xst[:, dbp:dbp + 2, MCH * sc:MCH * (sc + 1)],
                                    start=(x == 0), stop=(x == 1), perf_mode=DR,
                                )
                            nc.scalar.activation(
                                at[:, kb:kb + 1, MCH * sc:MCH * (sc + 1)], ps[:],
                                AFT.Identity, scale=ATS / W2S, bias=qbk[:, kb:kb + 1],
                            )
                    # V[s, v] = sum_d X_sel[s, d] Wv[d, v]  (+ bv broadcast)
                    for i in range(NSLOT):
                        ps = aps.tile([P, MCH], df32, tag="avp")
                        for x, dbp in enumerate((0, 2)):
                            nc.tensor.matmul(
                                ps[:],
                                xst[:, dbp:dbp + 2, P * i:P * (i + 1)],
                                wv[:, dbp:dbp + 2, :],
                                start=(x == 0), stop=(x == 1), perf_mode=DR,
                            )
                        if with_bv:
                            tv = sp.tile([P, D], dbf, tag="tv")
                            nc.vector.tensor_scalar_mul(tv[:], ps[:], 1.0 / WVS)
                            nc.vector.tensor_add(vsf[:, i:i + 1, :], tv[:], bvt[:])
                        else:
                            nc.vector.tensor_scalar_mul(vsf[:, i:i + 1, :], ps[:], 1.0 / WVS)

                # Phase B: per slot logits -> exp -> row sums -> normalized fp8
                for i in range(NSLOT):
                    c0 = 256 * i
                    chunks = [(t0, min(ECH, T - t0)) for t0 in range(c0, T, ECH)]
                    nch = len(chunks)
                    sums = sp.tile([P, 4], df32, tag="sums")
                    pexp = ep.tile([P, T], dbf, tag="pexp")
                    for g0 in range(0, nch, 2):
                        grp = chunks[g0:g0 + 2]
                        pss = []
                        for _g in range(len(grp)):
                            lgt = lp.tile([P, ECH], df32, tag="lg", name=f"lg{_g}")
                            pss.append(lgt)
                        # kbp outer so the at stationary loads once per 2 chunks
                        for x, kbp in enumerate((0, 2)):
                            for el, (ps, (t0, w)) in enumerate(zip(pss, grp)):
                                diag = (g0 + el == 0)
                                for h0 in range(0, w, MCH):
                                    hw = min(MCH, w - h0)
                                    nc.tensor.matmul(
                                        ps[:, h0:h0 + hw],
                                        at[:, kbp:kbp + 2, P * i:P * (i + 1)],
                                        xt[:, kbp:kbp + 2, t0 + h0:t0 + h0 + hw],
                                        start=(x == 0),
                                        stop=(x == 1) and not (diag and h0 == 0),
                                        perf_mode=DR,
                                    )
                                if x == 1 and diag:
                                    # diagonal block: add -240 to masked entries
                                    # (DoubleRow with a zero second plane)
                                    mw = min(MCH, w)
                                    nc.tensor.matmul(
                                        ps[:, 0:mw], idt[:, :, 0:P],
                                        maskn[:, :, 0:mw],
                                        start=False, stop=True, perf_mode=DR,
                                    )
                        for el, (ps, (t0, w)) in enumerate(zip(pss, grp)):
                            e = g0 + el
                            nc.scalar.activation(
                                pexp[:, t0:t0 + w], ps[:, 0:w],
                                AFT.Exp, scale=1.0 / ATS, bias=cs[:, i:i + 1],
                                accum_out=sums[:, e:e + 1],
                            )
                    den = sp.tile([P, 1], df32, tag="den")
                    nc.vector.tensor_reduce(den[:], sums[:, 0:nch],
                                            axis=mybir.AxisListType.X, op=ALU.add)
                    r2 = sp.tile([P, 1], df32, tag="r2")
                    nc.vector.reciprocal(r2[:], den[:])
                    eng = nc.gpsimd if i in GP_SLOTS else nc.vector
                    eng.tensor_scalar_mul(pf8[:, i:i + 1, c0:T], pexp[:, c0:T], r2[:])

                    # Phase C interleaved: t-blocks 2i, 2i+1 need only slots 0..i,
                    # so their AV matmuls can fill PE gaps during later softmaxes.
                    for tau in (2 * i, 2 * i + 1):
                        n = i + 1  # slots with window start <= 128*tau
                        jps = list(range(0, n, 2))  # odd n: last pair hits zero pad
                        ps = cp.tile([P, D], df32, tag="avp")
                        for idx, jp in enumerate(jps):
                            nc.tensor.matmul(
                                ps[:],
                                pf8[:, jp:jp + 2, P * tau:P * (tau + 1)],
                                vsf[:, jp:jp + 2, :],
                                start=(idx == 0), stop=(idx == len(jps) - 1),
                                perf_mode=DR,
                            )
                        st = sp.tile([P, D], dbf, tag="st")
                        nc.vector.tensor_scalar_mul(st[:], ps[:], INV_SQRT_K)
                        nc.sync.dma_start(OUT[P * tau:P * (tau + 1), :], st[:])

            for rep in range(reps):
                one_rep(rep)

            if OUT2 is not None:
                fin = sp.tile([P, 4], df32, tag="fin")
                nc.gpsimd.memset(fin[:], 0.0)
                nc.sync.dma_start(OUT2[:], fin[:])

    nc.compile()
    return nc


_PROGRAMS = {}


def _get_program(with_bv=False):
    if with_bv not in _PROGRAMS:
        _PROGRAMS[with_bv] = _build_program(with_bv=with_bv)
    return _PROGRAMS[with_bv]


def _f8(x, scale=1.0):
    return np.clip(np.asarray(x, np.float32) * scale, -240.0, 240.0).astype(f8)


def _core_inputs(X, W2_b, Wv_b, QBK_h, BV_b, masks, wkbq, bkbq, b, h):
    """Per-core input map for core (b, h)."""
    Xb = X[b]
    XTb = _f8(np.ascontiguousarray(Xb.T))
    sel = Xb.reshape(16, 2, P, D)[:, h].reshape(2048, D)
    XSTb = _f8(np.ascontiguousarray(sel.T))
    # per-key-row logit bias c_s = (x_s Wk).bq + bk.bq, [2048] -> [128, 16]
    cvec = sel.astype(np.float64) @ wkbq + bkbq
    CS_h = np.ascontiguousarray(cvec.reshape(NSLOT, P).T).astype(np.float32)
    m = {
        "XT": XTb, "XST": XSTb,
        "W2": W2_b, "WV": Wv_b,
        "QBK": QBK_h, "CS": CS_h,
        "MASKN": masks[h], "IDT": masks[2],
    }
    if BV_b is not None:
        m["BVT"] = BV_b
    return m


def _prep_shared(Wk, bk, Wq, bq, Wv, bv):
    Wk64 = np.asarray(Wk, np.float64)
    Wq64 = np.asarray(Wq, np.float64)
    W2_b = _f8(Wk64 @ Wq64.T, W2S)                               # lhsT for A^T
    Wv_b = _f8(np.asarray(Wv), WVS)
    qbk = Wq64 @ np.asarray(bk, np.float64)                      # [512]
    QBK_h = np.ascontiguousarray(qbk.reshape(4, P).T * ATS).astype(np.float32)
    wkbq = Wk64 @ np.asarray(bq, np.float64)                     # [512]
    bkbq = float(np.asarray(bk, np.float64) @ np.asarray(bq, np.float64))
    if np.any(np.asarray(bv) != 0):
        BV_b = np.tile(np.asarray(bv).astype(bf16)[None, :], (P, 1))
    else:
        BV_b = None
    s_loc = np.arange(P)[:, None]
    t_loc = np.arange(MCH)[None, :]
    masks = []  # [maskn_h0 | zeros, maskn_h1 | zeros, identity | zeros]
    for h in range(2):
        mn = np.zeros((P, 2 * MCH), np.float32)
        mn[:, :MCH] = np.where(t_loc >= P * h + s_loc, 0.0, -240.0)
        masks.append(mn.astype(f8))
    idz = np.zeros((P, 2 * P), np.float32)
    idz[:, :P] = np.eye(P)
    masks.append(idz.astype(f8))
    return W2_b, Wv_b, QBK_h, BV_b, masks, wkbq, bkbq


def kernel(minibatch, Wk, bk, Wq, bq, Wv, bv):
    X = np.asarray(minibatch, dtype=np.float32)
    shared = _prep_shared(Wk, bk, Wq, bq, Wv, bv)
    nc = _get_program(with_bv=shared[3] is not None)
    in_maps = [
        _core_inputs(X, *shared, b, h)
        for b in range(B) for h in range(2)
    ]
    last_exc = None
    for attempt in range(4):
        try:
            res = run_bass_kernel_spmd(nc, in_maps, list(range(2 * B)))
        except Exception as exc:  # transient device wedge — retry
            last_exc = exc
            continue
        out = X.copy()
        for b in range(B):
            out[b] += res.results[2 * b]["OUT"].astype(np.float32)
            out[b] += res.results[2 * b + 1]["OUT"].astype(np.float32)
        # transient device faults can surface as NaN/garbage — retry
        if not np.isnan(out).any() and np.abs(out).max() < 1e4:
            return out
    if last_exc is not None:
        raise last_exc
    return out
